# revision 21
# baseline (speedup 1.0000x reference)
"""Trainium2 Bass kernel for a 2-branch, 2-layer GCN (nn_Net_7172595384447).

v2 strategy (8 NeuronCores, SPMD):
  - Host relabels nodes by descending in-degree; tiles of 128 nodes are
    assigned round-robin to cores. Per dst tile, incoming edges fill k_t
    slot-chunks of 128 (slot i of lane d = edge #i into dst d; pads point
    at a zeroed table row with positive int16 encoding, so no +1 guard
    chunk is needed as long as the call's last slot encodes >= 0).
  - Phase A: own = dinv^2*(x@W1|dat@Wd1) kept in SBUF; table rows
    hs1 = dinv*h in bf16; chunked AllGather (7 chunks) overlaps the loop.
  - Phase B: per dst tile, dma_gather of slot rows from hs1_full;
    identity-matmul accumulate into PSUM; t1 = dinv*relu(dinv*agg + own);
    t2 = t1@Wcomb (16 cols) -> second table; chunked AllGather overlaps.
  - Phase C: same gather structure over t2_full with the SAME idx table;
    z = dinv*agg2 + dinv*t2_own; log_softmax; host unpermutes rows.
"""

import numpy as np
import ml_dtypes

import concourse.bass as bass
import concourse.mybir as mybir
import concourse.tile as tile
from concourse import bacc
from concourse.bass_utils import run_bass_kernel_spmd
from concourse.masks import make_identity

NCORES = 8
N = 50000
FX = 512
FD = 64
TILES = 49                  # tiles per core
SH_PAD = TILES * 128        # 6272 padded shard rows per core
NT = SH_PAD * NCORES        # 50176 padded table rows
NCHUNK = 7                  # allgather chunks
CH_ROWS = SH_PAD // NCHUNK  # 896 shard rows per AG chunk
CH_TILES = TILES // NCHUNK  # 7 tiles per AG chunk
BIAS = 32768                # int16 index bias
H1 = 96                     # feature cols (64 + 32)
H1P = 128                   # padded table cols (256B rows)

_CACHE = {}


def _host_prep(edge_index):
    src = np.asarray(edge_index[0], dtype=np.int64)
    dst = np.asarray(edge_index[1], dtype=np.int64)

    deg = np.bincount(dst, minlength=N).astype(np.int64) + 1  # incl self-loop
    dinv = (1.0 / np.sqrt(deg.astype(np.float64))).astype(np.float32)

    # degree-sorted relabel: rank r <-> node order[r]
    order = np.argsort(-deg, kind="stable")
    rank_of = np.empty(N, dtype=np.int64)
    rank_of[order] = np.arange(N)

    # rank -> (core, local); table row in chunk-major AllGather layout
    r_all = np.arange(NT, dtype=np.int64)
    g = r_all // 128
    core_of_rank = g % NCORES
    local_of_rank = (g // NCORES) * 128 + r_all % 128
    j = local_of_rank // CH_ROWS
    trow_of_rank = (j * (CH_ROWS * NCORES) + core_of_rank * CH_ROWS
                    + local_of_rank % CH_ROWS)
    # pick a pad rank whose table row encodes non-negative in int16
    pad_trows = trow_of_rank[N:]
    pos = pad_trows[pad_trows >= BIAS]
    assert len(pos) > 0, "no pad row with trow >= BIAS"
    zero_trow = int(pos[0])

    # per-core real-node lists and their local positions
    nodes_of_core = []
    pos_of_node = np.empty(N, dtype=np.int64)
    for c in range(NCORES):
        ranks = np.where(core_of_rank[:N] == c)[0]
        nodes = order[ranks]
        loc = local_of_rank[ranks]
        nodes_of_core.append((nodes, loc))
        pos_of_node[nodes] = c * SH_PAD + loc

    # edge slot assignment, grouped by (core, tile, lane)
    r_dst = rank_of[dst]
    e_core = core_of_rank[r_dst]
    e_tile = local_of_rank[r_dst] // 128
    e_lane = local_of_rank[r_dst] % 128
    e_srow = trow_of_rank[rank_of[src]]

    indeg = np.bincount(r_dst, minlength=NT)
    k_ct = np.zeros((NCORES, TILES), dtype=np.int64)
    np.maximum.at(k_ct, (core_of_rank, local_of_rank // 128), indeg)
    k_t = np.maximum(k_ct.max(axis=0), 1)
    # lane 127 of every (core, tile) must end with a pad slot so nothing
    # real can be int16-negative at the end of the gather call; in-degrees
    # are sorted descending within each tile, so only bump when the whole
    # tile is flat.
    lane127 = indeg[:NT].reshape(-1, 128)[:, 127]          # per 128-group
    l127_ct = np.zeros((NCORES, TILES), dtype=np.int64)
    grp = np.arange(NT // 128)
    np.maximum.at(l127_ct, (grp % NCORES, grp // NCORES), lane127)
    k_t = np.where(l127_ct.max(axis=0) >= k_t, k_t + 1, k_t)

    tile_off = np.concatenate([[0], np.cumsum(k_t * 128)])
    tot_slots = int(tile_off[-1])

    perm = np.lexsort((e_lane, e_tile, e_core))
    sc, st_, sl, sr = e_core[perm], e_tile[perm], e_lane[perm], e_srow[perm]
    grp2 = (sc * TILES + st_) * 128 + sl
    lo = np.searchsorted(grp2, np.arange(NCORES * TILES * 128))
    rank_in_lane = np.arange(len(grp2)) - lo[grp2]
    slot_pos = tile_off[st_] + rank_in_lane * 128 + sl

    idx_all = []
    for c in range(NCORES):
        slots = np.full(tot_slots, zero_trow, dtype=np.int64)
        m = sc == c
        slots[slot_pos[m]] = sr[m]
        idx16 = (slots - BIAS).astype(np.int16).reshape(tot_slots // 16, 16).T
        idx_all.append(np.tile(idx16, (8, 1)).copy())

    # per-core dinv, dinv^2 in [128, TILES] layout (pad lanes -> 1.0)
    dinv_ct = np.zeros((NCORES, 128, TILES), np.float32)
    for c in range(NCORES):
        nodes, loc = nodes_of_core[c]
        dv = np.ones(SH_PAD, np.float32)
        dv[loc] = dinv[nodes]
        dinv_ct[c] = dv.reshape(TILES, 128).T

    return k_t, idx_all, dinv_ct, nodes_of_core, pos_of_node


def _build(k_t):
    dt = mybir.dt
    f32 = dt.float32
    bf16 = dt.bfloat16
    tot_c = int(sum(k_t))

    nc = bacc.Bacc("TRN2", target_bir_lowering=False, debug=False,
                   num_devices=NCORES)
    xT = nc.dram_tensor("xT", [FX, SH_PAD], bf16, kind="ExternalInput")
    datT = nc.dram_tensor("datT", [FD, SH_PAD], bf16, kind="ExternalInput")
    W1 = nc.dram_tensor("W1", [FX, 64], bf16, kind="ExternalInput")
    Wd1 = nc.dram_tensor("Wd1", [FD, 32], bf16, kind="ExternalInput")
    Wcomb = nc.dram_tensor("Wcomb", [H1, 16], bf16, kind="ExternalInput")
    dinv_t = nc.dram_tensor("dinv_t", [128, TILES], f32, kind="ExternalInput")
    dinv2_t = nc.dram_tensor("dinv2_t", [128, TILES], f32, kind="ExternalInput")
    idx_d = nc.dram_tensor("idx", [128, tot_c * 8], dt.int16,
                           kind="ExternalInput")
    y = nc.dram_tensor("y", [SH_PAD, 16], f32, kind="ExternalOutput")

    hs1_shard = nc.dram_tensor("hs1_shard", [SH_PAD, H1P], bf16,
                               kind="Internal")
    t2_shard = nc.dram_tensor("t2_shard", [SH_PAD, H1P], bf16,
                              kind="Internal")
    hs1_full = nc.dram_tensor("hs1_full", [NT, H1P], bf16,
                              kind="Internal", addr_space="Shared")
    t2_full = nc.dram_tensor("t2_full", [NT, H1P], bf16,
                             kind="Internal", addr_space="Shared")

    with tile.TileContext(nc) as tc:
        with tc.tile_pool(name="const", bufs=1) as constp:
            idx_t = constp.tile([128, tot_c * 8], dt.int16)
            nc.sync.dma_start(out=idx_t[:], in_=idx_d[:])
            dinv_s = constp.tile([128, TILES], f32)
            nc.sync.dma_start(out=dinv_s[:], in_=dinv_t[:])
            dinv2_s = constp.tile([128, TILES], f32)
            nc.sync.dma_start(out=dinv2_s[:], in_=dinv2_t[:])
            wc_s = constp.tile([H1, 16], bf16)
            nc.sync.dma_start(out=wc_s[:], in_=Wcomb[:])
            ident = constp.tile([128, 128], bf16)
            make_identity(nc, ident[:])

            # per-tile self terms kept resident
            own1 = constp.tile([128, TILES, H1], f32)       # dinv^2 * h
            own2 = constp.tile([128, TILES, 16], f32)       # dinv * t2

            # ---------------- Phase A ----------------
            with tc.tile_pool(name="phA", bufs=3) as pa, \
                 tc.tile_pool(name="phA_w", bufs=1) as paw, \
                 tc.tile_pool(name="psA", bufs=4, space="PSUM") as psa:
                w1_s = paw.tile([128, FX // 128, 64], bf16)
                for kk in range(FX // 128):
                    nc.sync.dma_start(out=w1_s[:, kk, :],
                                      in_=W1[kk * 128:(kk + 1) * 128, :])
                wd1_s = paw.tile([FD, 32], bf16)
                nc.sync.dma_start(out=wd1_s[:], in_=Wd1[:])
                xT_s = []
                for kk in range(FX // 128):
                    stl = paw.tile([128, SH_PAD], bf16, tag=f"xT{kk}")
                    nc.sync.dma_start(out=stl[:],
                                      in_=xT[kk * 128:(kk + 1) * 128, :])
                    xT_s.append(stl)
                datT_s = paw.tile([FD, SH_PAD], bf16)
                nc.sync.dma_start(out=datT_s[:], in_=datT[:])

                for t in range(TILES):
                    ps = psa.tile([128, H1], f32, space="PSUM", tag="psA")
                    for kk in range(FX // 128):
                        nc.tensor.matmul(
                            out=ps[:, 0:64],
                            lhsT=xT_s[kk][:, t * 128:(t + 1) * 128],
                            rhs=w1_s[:, kk, :],
                            start=(kk == 0), stop=(kk == FX // 128 - 1))
                    nc.tensor.matmul(
                        out=ps[:, 64:96],
                        lhsT=datT_s[:, t * 128:(t + 1) * 128],
                        rhs=wd1_s[:],
                        start=True, stop=True)
                    nc.vector.tensor_scalar_mul(
                        own1[:, t, :], ps[:], dinv2_s[:, t:t + 1])
                    hso = pa.tile([128, H1P], bf16, tag="hs1o")
                    nc.vector.tensor_scalar_mul(
                        hso[:, :H1], ps[:], dinv_s[:, t:t + 1])
                    nc.sync.dma_start(
                        out=hs1_shard[t * 128:(t + 1) * 128, :], in_=hso[:])
                    if (t + 1) % CH_TILES == 0:
                        jc = t // CH_TILES
                        nc.gpsimd.collective_compute(
                            "AllGather", mybir.AluOpType.bypass,
                            replica_groups=[list(range(NCORES))],
                            ins=[hs1_shard[jc * CH_ROWS:(jc + 1) * CH_ROWS, :]],
                            outs=[hs1_full[jc * CH_ROWS * NCORES:
                                           (jc + 1) * CH_ROWS * NCORES, :]])

            # ---------------- Phase B ----------------
            groups = [tuple(u for u in range(4 * p, 4 * p + 4) if u < TILES)
                      for p in range((TILES + 3) // 4)]
            with tc.tile_pool(name="phB", bufs=3) as pb, \
                 tc.tile_pool(name="psB", bufs=4, space="PSUM") as psb, \
                 tc.tile_pool(name="psBt", bufs=2, space="PSUM") as psbt, \
                 tc.tile_pool(name="psB2", bufs=2, space="PSUM") as psb2:
                off = 0
                for grp in groups:
                    ktot = int(sum(k_t[u] for u in grp))
                    msg = pb.tile([128, ktot, H1P], bf16, tag="msg")
                    nc.gpsimd.dma_gather(
                        out_ap=msg[:], in_ap=hs1_full[BIAS:, :],
                        idxs_ap=idx_t[:, off * 8:(off + ktot) * 8],
                        num_idxs=ktot * 128, num_idxs_reg=ktot * 128,
                        elem_size=H1P, single_packet=False)
                    off += ktot
                    sub = 0
                    for t in grp:
                      kt = int(k_t[t])
                      ps1 = psb.tile([128, H1], f32, space="PSUM", tag="ps1")
                      for jj in range(kt):
                        nc.tensor.matmul(
                            out=ps1[:], lhsT=ident[:],
                            rhs=msg[:, sub + jj, :H1],
                            start=(jj == 0), stop=(jj == kt - 1))
                      sub += kt
                      v = pb.tile([128, H1], f32, tag="v")
                      nc.vector.scalar_tensor_tensor(
                          out=v[:], in0=ps1[:], scalar=dinv_s[:, t:t + 1],
                          in1=own1[:, t, :],
                          op0=mybir.AluOpType.mult, op1=mybir.AluOpType.add)
                      t1c = pb.tile([128, H1], bf16, tag="t1c")
                      nc.vector.tensor_scalar(
                          out=t1c[:], in0=v[:], scalar1=0.0,
                          scalar2=dinv_s[:, t:t + 1],
                          op0=mybir.AluOpType.max, op1=mybir.AluOpType.mult)
                      pst = psbt.tile([H1, 128], bf16, space="PSUM", tag="pst")
                      nc.tensor.transpose(out=pst[:], in_=t1c[:],
                                          identity=ident[:])
                      t1T = pb.tile([H1, 128], bf16, tag="t1T")
                      nc.vector.tensor_copy(t1T[:], pst[:])
                      ps4 = psb2.tile([128, 16], f32, space="PSUM", tag="ps4")
                      nc.tensor.matmul(out=ps4[:], lhsT=t1T[:], rhs=wc_s[:],
                                       start=True, stop=True)
                      nc.vector.tensor_scalar_mul(
                          own2[:, t, :], ps4[:], dinv_s[:, t:t + 1])
                      t2b = pb.tile([128, H1P], bf16, tag="t2b")
                      nc.vector.tensor_copy(t2b[:, :16], ps4[:])
                      nc.sync.dma_start(
                          out=t2_shard[t * 128:(t + 1) * 128, :], in_=t2b[:])
                      if (t + 1) % CH_TILES == 0:
                        jc = t // CH_TILES
                        nc.gpsimd.collective_compute(
                            "AllGather", mybir.AluOpType.bypass,
                            replica_groups=[list(range(NCORES))],
                            ins=[t2_shard[jc * CH_ROWS:(jc + 1) * CH_ROWS, :]],
                            outs=[t2_full[jc * CH_ROWS * NCORES:
                                          (jc + 1) * CH_ROWS * NCORES, :]])

            # ---------------- Phase C ----------------
            with tc.tile_pool(name="phC", bufs=4) as pc_, \
                 tc.tile_pool(name="psC", bufs=6, space="PSUM") as psc:
                off = 0
                for grp in groups:
                    ktot = int(sum(k_t[u] for u in grp))
                    msg = pc_.tile([128, ktot, H1P], bf16, tag="msg2")
                    nc.gpsimd.dma_gather(
                        out_ap=msg[:], in_ap=t2_full[BIAS:, :],
                        idxs_ap=idx_t[:, off * 8:(off + ktot) * 8],
                        num_idxs=ktot * 128, num_idxs_reg=ktot * 128,
                        elem_size=H1P, single_packet=False)
                    off += ktot
                    sub = 0
                    for t in grp:
                      kt = int(k_t[t])
                      ps5 = psc.tile([128, 16], f32, space="PSUM", tag="ps5")
                      for jj in range(kt):
                        nc.tensor.matmul(
                            out=ps5[:], lhsT=ident[:],
                            rhs=msg[:, sub + jj, :16],
                            start=(jj == 0), stop=(jj == kt - 1))
                      sub += kt
                      z = pc_.tile([128, 16], f32, tag="z")
                      nc.vector.scalar_tensor_tensor(
                          out=z[:], in0=ps5[:], scalar=dinv_s[:, t:t + 1],
                          in1=own2[:, t, :],
                          op0=mybir.AluOpType.mult, op1=mybir.AluOpType.add)
                      # logits are O(10) max, far below exp overflow in f32,
                      # so skip the max-subtraction stabilizer
                      ex = pc_.tile([128, 16], f32, tag="ex")
                      nc.scalar.activation(
                          ex[:], z[:], mybir.ActivationFunctionType.Exp)
                      s = pc_.tile([128, 1], f32, tag="s")
                      nc.vector.reduce_sum(s[:], ex[:],
                                           axis=mybir.AxisListType.X)
                      ls = pc_.tile([128, 1], f32, tag="ls")
                      nc.scalar.activation(
                          ls[:], s[:], mybir.ActivationFunctionType.Ln)
                      ot = pc_.tile([128, 16], f32, tag="ot")
                      nc.vector.tensor_scalar(
                          out=ot[:], in0=z[:], scalar1=ls[:, :1],
                          scalar2=None, op0=mybir.AluOpType.subtract)
                      nc.sync.dma_start(
                          out=y[t * 128:(t + 1) * 128, :], in_=ot[:])

    nc.compile()
    return nc


def kernel(x, dat, edge_index, W1, b1, W2, b2, Wd1, bd1, Wd2, bd2):
    x = np.asarray(x, dtype=np.float32)
    dat = np.asarray(dat, dtype=np.float32)
    for b in (b1, bd1, b2, bd2):
        assert not np.any(np.asarray(b)), "nonzero bias unsupported"
    k_t, idx_all, dinv_ct, nodes_of_core, pos_of_node = _host_prep(
        np.asarray(edge_index))

    key = tuple(int(v) for v in k_t)
    if key not in _CACHE:
        _CACHE[key] = _build(k_t)
    nc = _CACHE[key]

    bf = ml_dtypes.bfloat16
    W1f = np.asarray(W1, np.float32).astype(bf)
    Wd1f = np.asarray(Wd1, np.float32).astype(bf)
    Wcomb = np.concatenate([0.2 * np.asarray(W2, np.float32),
                            0.1 * np.asarray(Wd2, np.float32)],
                           axis=0).astype(bf)

    in_maps = []
    for c in range(NCORES):
        nodes, loc = nodes_of_core[c]
        xs = np.zeros((SH_PAD, FX), np.float32)
        ds = np.zeros((SH_PAD, FD), np.float32)
        xs[loc] = x[nodes]
        ds[loc] = dat[nodes]
        in_maps.append({
            "xT": np.ascontiguousarray(xs.T).astype(bf),
            "datT": np.ascontiguousarray(ds.T).astype(bf),
            "W1": W1f, "Wd1": Wd1f, "Wcomb": Wcomb,
            "dinv_t": dinv_ct[c],
            "dinv2_t": dinv_ct[c] * dinv_ct[c],
            "idx": idx_all[c],
        })

    res = run_bass_kernel_spmd(nc, in_maps, core_ids=list(range(NCORES)))
    yfull = np.concatenate([res.results[c]["y"] for c in range(NCORES)],
                           axis=0)
    return yfull[pos_of_node].astype(np.float32)
